# revision 12
# baseline (speedup 1.0000x reference)
"""Causal multi-head attention with RoPE on Trainium2, 8 NeuronCores.

Sharding: head-parallel (tensor parallel). 16 heads / 8 cores = 2 heads per
core. Each core computes q/k/v projections for its 2 heads (128 of the 1024
hidden dims), flash-style causal attention in transposed-score layout
(keys on partitions, queries on free dim -- softmax denominator comes from an
appended ones-column in V so no cross-partition reduction is needed), and a
row-parallel slice of the output projection. Cores emit partial (D,S) f32
outputs that the host sums and transposes.

Self-contained: hardcodes shapes B=1, S=4096, D=1024, H=16, hd=64.
"""

import sys

if "/opt/trn_rl_repo" not in sys.path:
    sys.path.insert(0, "/opt/trn_rl_repo")

import numpy as np

S = 4096
D = 1024
H = 16
HD = 64
NCORE = 8
P = 128
QB = 512          # query block width (psum strip per head)
NQB = S // QB     # 8
KC = 128          # key chunk (psum partitions per scores tile)
THETA = 10000.0

MM_DTYPE = "bf16"  # "f32r" or "bf16"
DEBUG_TAPS = False
_NC_CACHE = {}


def _build_nc():
    import concourse.bacc as bacc
    import concourse.mybir as mybir
    from concourse.tile import TileContext
    from contextlib import ExitStack

    F32 = mybir.dt.float32
    MMDT = mybir.dt.float32r if MM_DTYPE == "f32r" else mybir.dt.bfloat16
    EXP = mybir.ActivationFunctionType.Exp

    nc = bacc.Bacc("TRN2", target_bir_lowering=False)

    xT = nc.dram_tensor("xT", [D, S], MMDT, kind="ExternalInput")
    wq = nc.dram_tensor("wq", [D, P], MMDT, kind="ExternalInput")
    wk = nc.dram_tensor("wk", [D, P], MMDT, kind="ExternalInput")
    wv = nc.dram_tensor("wv", [D, P], MMDT, kind="ExternalInput")
    wo = nc.dram_tensor("wo", [P, D], MMDT, kind="ExternalInput")
    cs = nc.dram_tensor("cs", [P, S], F32, kind="ExternalInput")
    sn = nc.dram_tensor("sn", [P, S], F32, kind="ExternalInput")
    cst = nc.dram_tensor("cst", [P, 3 * P], MMDT, kind="ExternalInput")
    y = nc.dram_tensor("y", [D, S], F32, kind="ExternalOutput")
    if DEBUG_TAPS:
        dq = nc.dram_tensor("dq", [P, S], F32, kind="ExternalOutput")
        dk_ = nc.dram_tensor("dk", [P, S], F32, kind="ExternalOutput")
        dv = nc.dram_tensor("dv", [P, 2 * (S // KC) * (HD + 1)], F32, kind="ExternalOutput")
        dl = nc.dram_tensor("dl", [2, S], F32, kind="ExternalOutput")
        dlr = nc.dram_tensor("dlr", [2, S], F32, kind="ExternalOutput")
        dpt = nc.dram_tensor("dpt", [P, 2 * QB], F32, kind="ExternalOutput")
        don = nc.dram_tensor("don", [P, S], F32, kind="ExternalOutput")

    xTr = xT.rearrange("(o p) s -> p o s", p=P)   # [128, 8, 4096]
    wqr = wq.rearrange("(o p) m -> p o m", p=P)   # [128, 8, 128]
    wkr = wk.rearrange("(o p) m -> p o m", p=P)
    wvr = wv.rearrange("(o p) m -> p o m", p=P)

    with TileContext(nc) as tc, ExitStack() as ctx:
        con = ctx.enter_context(tc.tile_pool(name="con", bufs=1))
        xp = ctx.enter_context(tc.tile_pool(name="xp", bufs=10))
        tp = ctx.enter_context(tc.tile_pool(name="tp", bufs=2))
        ptp = ctx.enter_context(tc.tile_pool(name="ptp", bufs=5))
        onp_ = ctx.enter_context(tc.tile_pool(name="onp", bufs=2))
        rlp = ctx.enter_context(tc.tile_pool(name="rlp", bufs=2))
        fsp = ctx.enter_context(tc.tile_pool(name="fsp", bufs=4))
        pp = ctx.enter_context(tc.tile_pool(name="pp", bufs=2, space="PSUM"))
        scp = ctx.enter_context(tc.tile_pool(name="scp", bufs=1, space="PSUM"))
        oap = ctx.enter_context(tc.tile_pool(name="oap", bufs=1, space="PSUM"))
        obp = ctx.enter_context(tc.tile_pool(name="obp", bufs=1, space="PSUM"))

        # ---- constants / weights ----
        wq_sb = con.tile([P, 8, P], MMDT)
        nc.sync.dma_start(out=wq_sb, in_=wqr)
        wk_sb = con.tile([P, 8, P], MMDT)
        nc.sync.dma_start(out=wk_sb, in_=wkr)
        wv_sb = con.tile([P, 8, P], MMDT)
        nc.sync.dma_start(out=wv_sb, in_=wvr)
        wo_sb = con.tile([P, D], MMDT)
        nc.sync.dma_start(out=wo_sb, in_=wo[:, :])
        cs_sb = con.tile([P, S], F32)
        nc.sync.dma_start(out=cs_sb, in_=cs[:, :])
        sn_sb = con.tile([P, S], F32)
        nc.sync.dma_start(out=sn_sb, in_=sn[:, :])
        cst_sb = con.tile([P, 3, P], MMDT)
        nc.sync.dma_start(out=cst_sb, in_=cst.rearrange("p (t m) -> p t m", t=3))
        pswap_sb = cst_sb[:, 0, :]
        ident_sb = cst_sb[:, 1, :]
        tri_sb = cst_sb[:, 2, :]

        # v in natural layout [key, hd] per head, with a 65th ones column
        # (the ones column makes attnV also accumulate the softmax denom).
        vna = [
            con.tile([P, S // KC, HD + 1], MMDT, tag=f"vna{h}", name=f"vna{h}")
            for h in (0, 1)
        ]
        for h in (0, 1):
            nc.vector.memset(vna[h][:, :, HD : HD + 1], 1.0)

        # roped q^T, k^T for both heads: rows 0:64 head A, 64:128 head B
        qTr = con.tile([P, S], MMDT, tag="qTr")
        kTr = con.tile([P, S], MMDT, tag="kTr")

        # ---- phase A: projections + rope + v transpose ----
        for st in range(NQB):
            sl = slice(st * QB, (st + 1) * QB)
            xts = []
            for dk in range(8):
                xt = xp.tile([P, QB], MMDT, tag="x")
                nc.sync.dma_start(out=xt, in_=xTr[:, dk, sl])
                xts.append(xt)

            for wsb, dstT in ((wq_sb, qTr), (wk_sb, kTr)):
                acc = pp.tile([P, QB], F32, tag="acc")
                for dk in range(8):
                    nc.tensor.matmul(
                        acc,
                        wsb[:, dk, :],
                        xts[dk],
                        start=(dk == 0),
                        stop=(dk == 7),
                    )
                raw = tp.tile([P, QB], MMDT, tag="raw")
                nc.vector.tensor_copy(out=raw, in_=acc)
                sw = pp.tile([P, QB], F32, tag="sw")
                nc.tensor.matmul(
                    sw,
                    pswap_sb,
                    raw,
                    start=True,
                    stop=True,
                )
                t1 = tp.tile([P, QB], F32, tag="t1")
                nc.vector.tensor_mul(out=t1, in0=acc, in1=cs_sb[:, sl])
                t2 = tp.tile([P, QB], F32, tag="t2")
                nc.vector.tensor_mul(out=t2, in0=sw, in1=sn_sb[:, sl])
                nc.vector.tensor_add(out=dstT[:, sl], in0=t1, in1=t2)

            acc = pp.tile([P, QB], F32, tag="acc")
            for dk in range(8):
                nc.tensor.matmul(
                    acc,
                    wv_sb[:, dk, :],
                    xts[dk],
                    start=(dk == 0),
                    stop=(dk == 7),
                )
            vtmp = tp.tile([P, QB], MMDT, tag="raw")
            nc.vector.tensor_copy(out=vtmp, in_=acc)
            for sub in range(4):
                for h in (0, 1):
                    ptile = pp.tile([P, HD], MMDT, tag="sw")
                    nc.tensor.transpose(
                        ptile,
                        vtmp[h * HD : (h + 1) * HD, sub * KC : (sub + 1) * KC],
                        ident_sb[h * HD : (h + 1) * HD, h * HD : (h + 1) * HD],
                    )
                    nc.vector.tensor_copy(
                        out=vna[h][:, st * 4 + sub, 0:HD], in_=ptile
                    )

        if DEBUG_TAPS:
            dstg = con.tile([P, S], F32, tag="dstg")
            nc.vector.tensor_copy(out=dstg, in_=qTr)
            nc.sync.dma_start(out=dq[:, :], in_=dstg)
            dstg2 = con.tile([P, S], F32, tag="dstg2")
            nc.vector.tensor_copy(out=dstg2, in_=kTr)
            nc.sync.dma_start(out=dk_[:, :], in_=dstg2)
            dstg3 = con.tile([P, 2, S // KC, HD + 1], F32, tag="dstg3")
            nc.vector.tensor_copy(out=dstg3[:, 0], in_=vna[0])
            nc.vector.tensor_copy(out=dstg3[:, 1], in_=vna[1])
            nc.sync.dma_start(
                out=dv[:, :],
                in_=dstg3.rearrange("p a b c -> p (a b c)"),
            )

        # ---- phase B: flash attention (transposed scores) + output proj ----
        for qsb in range(NQB):
            nch = 4 * (qsb + 1)
            oA = oap.tile([P, QB], F32, tag="oA")
            oB = obp.tile([P, QB], F32, tag="oB")
            for c in range(nch):
                is_diag = (c // 4) == qsb
                off = (c % 4) * KC if is_diag else 0
                sp = scp.tile([P, 2, QB], F32, tag="sc")
                for h in (0, 1):
                    nc.tensor.matmul(
                        sp[:, h, off:QB],
                        kTr[h * HD : (h + 1) * HD, c * KC : (c + 1) * KC],
                        qTr[h * HD : (h + 1) * HD, qsb * QB + off : (qsb + 1) * QB],
                        start=True,
                        stop=True,
                        tile_position=(h * HD, 0),
                    )
                pt = ptp.tile([P, 2, QB], MMDT, tag="pt")
                nc.scalar.activation(
                    out=pt[:, :, off:QB], in_=sp[:, :, off:QB], func=EXP, scale=0.125
                )
                if is_diag:
                    for h in (0, 1):
                        nc.vector.tensor_mul(
                            out=pt[:, h, off : off + KC],
                            in0=pt[:, h, off : off + KC],
                            in1=tri_sb,
                        )
                if DEBUG_TAPS and qsb == 0 and c == 0:
                    ptstg = ptp.tile([P, 2, QB], F32, tag="ptstg")
                    nc.vector.tensor_copy(out=ptstg, in_=pt)
                    nc.sync.dma_start(
                        out=dpt.rearrange("p (a b) -> p a b", a=2), in_=ptstg
                    )
                for h, o in ((0, oA), (1, oB)):
                    nc.tensor.matmul(
                        o[0 : HD + 1, off:QB],
                        vna[h][:, c, :],
                        pt[:, h, off:QB],
                        start=(c == 0),
                        stop=(c == nch - 1),
                    )

            # normalize: out^T[d, q] / l[q]; l sits in row 64 of each o tile.
            # partition_broadcast and the custom-DVE recip only work on
            # base-partition-0 APs, so stage l to row 0 of an SBUF tile first,
            # broadcast the raw l, then recip on the [64, QB] broadcast.
            rlA = rlp.tile([1, QB], F32, tag="rlA")
            nc.vector.tensor_copy(out=rlA, in_=oA[HD : HD + 1, :])
            rlB = rlp.tile([1, QB], F32, tag="rlB")
            nc.vector.tensor_copy(out=rlB, in_=oB[HD : HD + 1, :])
            lbA = onp_.tile([HD, QB], F32, tag="lb")
            nc.gpsimd.partition_broadcast(lbA[0:HD, :], rlA[0:1, :])
            lbB = onp_.tile([HD, QB], F32, tag="lb")
            nc.gpsimd.partition_broadcast(lbB[0:HD, :], rlB[0:1, :])
            rlbA = onp_.tile([HD, QB], F32, tag="rlb")
            nc.vector.reciprocal_approx_fast(out=rlbA, in_=lbA)
            rlbB = onp_.tile([HD, QB], F32, tag="rlb")
            nc.vector.reciprocal_approx_fast(out=rlbB, in_=lbB)

            onT = onp_.tile([P, QB], MMDT, tag="onT")
            nc.vector.tensor_mul(out=onT[0:HD, :], in0=oA[0:HD, :], in1=rlbA[0:HD, :])
            oBn = onp_.tile([HD, QB], MMDT, tag="oBn")
            nc.vector.tensor_mul(out=oBn[0:HD, :], in0=oB[0:HD, :], in1=rlbB[0:HD, :])
            nc.sync.dma_start(out=onT[HD : 2 * HD, :], in_=oBn[0:HD, :])

            if DEBUG_TAPS:
                lrawA = rlp.tile([1, QB], F32, tag="lrawA")
                nc.vector.tensor_copy(out=lrawA, in_=oA[HD : HD + 1, :])
                nc.sync.dma_start(out=dlr[0:1, qsb * QB : (qsb + 1) * QB], in_=lrawA)
                lrawB = rlp.tile([1, QB], F32, tag="lrawB")
                nc.vector.tensor_copy(out=lrawB, in_=oB[HD : HD + 1, :])
                nc.sync.dma_start(out=dlr[1:2, qsb * QB : (qsb + 1) * QB], in_=lrawB)
                nc.sync.dma_start(out=dl[0:1, qsb * QB : (qsb + 1) * QB], in_=rlA)
                nc.sync.dma_start(out=dl[1:2, qsb * QB : (qsb + 1) * QB], in_=rlB)
                donstg = onp_.tile([P, QB], F32, tag="donstg")
                nc.vector.tensor_copy(out=donstg, in_=onT)
                nc.sync.dma_start(out=don[:, qsb * QB : (qsb + 1) * QB], in_=donstg)

            # final projection slice: y[j, qsb*QB:...] partial
            for jc in range(8):
                fpool = oap if jc % 2 == 0 else obp
                ftag = "oA" if jc % 2 == 0 else "oB"
                fp = fpool.tile([P, QB], F32, tag=ftag)
                nc.tensor.matmul(
                    fp,
                    wo_sb[:, jc * P : (jc + 1) * P],
                    onT,
                    start=True,
                    stop=True,
                )
                fs = fsp.tile([P, QB], F32, tag="fs")
                nc.vector.tensor_copy(out=fs, in_=fp)
                nc.sync.dma_start(
                    out=y[jc * P : (jc + 1) * P, qsb * QB : (qsb + 1) * QB], in_=fs
                )

    nc.compile()
    return nc


def _mm_np(a):
    if MM_DTYPE == "bf16":
        import ml_dtypes
        return np.ascontiguousarray(a).astype(ml_dtypes.bfloat16)
    return np.ascontiguousarray(a).astype(np.float32)


def _host_prep(x, token_positions, Wq, Wk, Wv, Wo):
    x = np.asarray(x, dtype=np.float32)
    pos = np.asarray(token_positions).astype(np.float32)
    Wq = np.asarray(Wq, dtype=np.float32)
    Wk = np.asarray(Wk, dtype=np.float32)
    Wv = np.asarray(Wv, dtype=np.float32)
    Wo = np.asarray(Wo, dtype=np.float32)

    xT = np.ascontiguousarray(x.reshape(S, D).T)  # [D, S]

    # rope tables in transposed layout, rows = head dims (both heads identical)
    freqs = (1.0 / THETA ** (np.arange(0, HD, 2, dtype=np.float32) / HD)).astype(
        np.float32
    )
    ang = pos[:, None] * freqs[None, :]          # [S, 32]
    cosv = np.cos(ang).astype(np.float32).T      # [32, S]
    sinv = np.sin(ang).astype(np.float32).T
    C64 = np.repeat(cosv, 2, axis=0)             # [64, S]
    S64 = np.empty((HD, S), dtype=np.float32)
    S64[0::2] = -sinv
    S64[1::2] = sinv
    C = np.tile(C64, (2, 1))                     # [128, S]
    Sg = np.tile(S64, (2, 1))

    pswap = np.zeros((P, P), dtype=np.float32)
    idx = np.arange(P)
    pswap[idx ^ 1, idx] = 1.0
    ident = np.eye(P, dtype=np.float32)
    tri = np.triu(np.ones((P, P), dtype=np.float32))
    cst = np.concatenate([pswap, ident, tri], axis=1)  # [128, 384]

    in_maps = []
    for c in range(NCORE):
        r = slice(c * P, (c + 1) * P)
        in_maps.append(
            {
                "xT": _mm_np(xT),
                "wq": _mm_np(Wq[r, :].T),
                "wk": _mm_np(Wk[r, :].T),
                "wv": _mm_np(Wv[r, :].T),
                "wo": _mm_np(Wo[:, r].T),
                "cs": C,
                "sn": Sg,
                "cst": _mm_np(cst),
            }
        )
    return in_maps


LAST_EXEC_NS = None
LAST_TRACE = None


def kernel(x, token_positions, Wq, Wk, Wv, Wo):
    global LAST_EXEC_NS, LAST_TRACE
    from concourse.bass_utils import run_bass_kernel_spmd

    if "nc" not in _NC_CACHE:
        _NC_CACHE["nc"] = _build_nc()
    nc = _NC_CACHE["nc"]

    in_maps = _host_prep(x, token_positions, Wq, Wk, Wv, Wo)
    res = run_bass_kernel_spmd(nc, in_maps, core_ids=list(range(NCORE)))
    LAST_EXEC_NS = res.exec_time_ns
    LAST_TRACE = (
        res.instructions_and_trace[1]
        if res.instructions_and_trace is not None
        else None
    )

    acc = np.zeros((D, S), dtype=np.float64)
    for r in res.results:
        acc += r["y"].astype(np.float64)
    out = acc.T.astype(np.float32).reshape(1, S, D)
    return out
